# revision 1
# baseline (speedup 1.0000x reference)
"""Multi-head causal self-attention on 8 Trainium2 NeuronCores.

Problem: B=4, S=2048, D=1024, H=16 heads (Dh=64), fp32, causal + key-padding
mask, out = softmax(mask(QK^T/sqrt(Dh))) V Wo^T with Q/K/V = x @ W*^T.

Sharding (data-parallel over batch x tensor-parallel over heads):
  core = 2*b + g  (b in 0..3, g in 0..1): batch b, head group g (8 heads).
  Each core computes its 8 heads' attention and a partial output projection
  through its row-slice of Wo; the host sums the two partials per batch
  (the "all-reduce" of the hint, done on host since outputs are gathered
  anyway).

Per-core kernel layout (everything f32; matmuls in float32r):
  - x^T [D, S] so projections contract D on partitions.
  - q^T, k^T stored [F=512, S] (head-major rows, 64 rows per head; heads
    2f/2f+1 live in partitions 0-63 / 64-127 of feature tile f).
  - scores computed TRANSPOSED per head: s^T[k, q] = k^T_tile.T @ q^T so the
    softmaxed tile feeds the AV matmul directly as the moving operand.
  - exp via ScalarE with fused scale (1/8) and per-key padding bias.
  - causal: only lower block-triangle computed; diagonal 128x128 blocks get a
    multiplicative triangular mask after exp; partial-width matmuls skip
    fully-masked column ranges.
  - V carries an appended ones-column per head so the AV matmul also yields
    the softmax denominators (row 64 of the [65, q] psum tile).
  - normalize: reciprocal on DVE, partition-broadcast on GpSimd, multiply on
    DVE straight into ctx^T tiles, which are the stationary operand of the
    output projection out[s, d] = ctx^T.T @ Wo_slice^T.
"""

import os
import numpy as np

import concourse.bass as bass
import concourse.mybir as mybir
import concourse.tile as tile
from concourse import bacc
from concourse.bass_utils import run_bass_kernel_spmd

P = 128
NEG = -1.0e30


def _round_f32r(a: np.ndarray) -> np.ndarray:
    """Round fp32 values to the PE's fp32r grid (11-bit mantissa,
    round-half-to-even at bit 12) so DMA-loaded tiles hold valid fp32r
    values. Matches walrus fp32_to_fp32r bit-exactly."""
    bits = np.ascontiguousarray(a, dtype=np.float32).view(np.uint32)
    low = bits & np.uint32(0xFFF)
    hi = bits & np.uint32(0xFFFFF000)
    add = (low > 0x800) | ((low == 0x800) & (((bits >> 12) & 1) == 1))
    return (hi + (add.astype(np.uint32) << 12)).view(np.float32)


class Cfg:
    def __init__(self, B=4, S=2048, D=1024, H=16, Dh=64, n_cores=8, qch=512,
                 mm_dtype="fp32r", reps=1):
        self.reps = reps
        self.B, self.S, self.D, self.H, self.Dh = B, S, D, H, Dh
        self.n_cores = n_cores
        self.groups = n_cores // B              # head groups (tensor-parallel)
        self.Hc = H // self.groups              # heads per core
        self.F = self.Hc * Dh                   # per-core q/k/v feature width
        self.qch = qch                          # q columns per score matmul
        self.nqc = S // qch                     # q chunks
        self.qt_per_ch = qch // P               # 128-row q tiles per chunk
        self.nt_s = S // P                      # key/seq tiles
        self.nt_d = D // P                      # contraction tiles (D)
        self.nt_f = self.F // P                 # feature tiles
        self.heads_per_ft = P // Dh             # heads packed per feature tile
        self.mm_dtype = mm_dtype

    @property
    def mdt(self):
        return {"fp32r": mybir.dt.float32r,
                "fp32": mybir.dt.float32,
                "bf16": mybir.dt.bfloat16}[self.mm_dtype]


def build_nc(cfg: Cfg):
    f32 = mybir.dt.float32
    mdt = cfg.mdt
    S, D, F, Dh = cfg.S, cfg.D, cfg.F, cfg.Dh
    QCH = cfg.qch

    nc = bacc.Bacc("TRN2", target_bir_lowering=False, debug=False,
                   num_devices=cfg.n_cores)

    xT = nc.dram_tensor("xT", [D, S], mdt, kind="ExternalInput").ap()
    wqT = nc.dram_tensor("wqT", [D, F], mdt, kind="ExternalInput").ap()
    wkT = nc.dram_tensor("wkT", [D, F], mdt, kind="ExternalInput").ap()
    wvT = nc.dram_tensor("wvT", [D, F], mdt, kind="ExternalInput").ap()
    woT = nc.dram_tensor("woT", [F, D], mdt, kind="ExternalInput").ap()
    pbias = nc.dram_tensor("pbias", [P, cfg.nt_s], f32, kind="ExternalInput").ap()
    out = nc.dram_tensor("out", [S, D], f32, kind="ExternalOutput").ap()

    Exp = mybir.ActivationFunctionType.Exp
    mult = mybir.AluOpType.mult

    with tile.TileContext(nc) as tc:
        with (
            tc.tile_pool(name="psA", bufs=(4 if cfg.qch <= 512 else 2),
                         space="PSUM") as psA,
            tc.tile_pool(name="psB", bufs=2, space="PSUM") as psB,
            tc.tile_pool(name="psC", bufs=(2 if cfg.qch <= 512 else 1),
                         space="PSUM") as psC,
            tc.tile_pool(name="sb_qT", bufs=cfg.nt_f) as sb_qT,
            tc.tile_pool(name="sb_kT", bufs=cfg.nt_f) as sb_kT,
            tc.tile_pool(name="sb_v", bufs=cfg.nt_s) as sb_v,
            tc.tile_pool(name="sb_misc", bufs=1) as sb_misc,
        ):
            # --- constants ---
            pb = sb_misc.tile([P, cfg.nt_s], f32, tag="pbias")
            nc.sync.dma_start(pb[:], pbias)
            # triangular keep-mask in [k(part), q(free)] coords: 1 where q>=k
            tri_f = sb_misc.tile([P, P], f32, tag="tri_f")
            nc.gpsimd.memset(tri_f[:], 1.0)
            nc.gpsimd.affine_select(
                out=tri_f[:], in_=tri_f[:],
                compare_op=mybir.AluOpType.is_ge, fill=0.0,
                base=0, channel_multiplier=-1, pattern=[[1, P]],
            )
            tri = sb_misc.tile([P, P], mdt, tag="tri")
            nc.vector.tensor_copy(tri[:], tri_f[:])
            ones_c = sb_misc.tile([P, 1], f32, tag="ones_c")
            nc.gpsimd.memset(ones_c[:], 1.0)

            qT_t = [sb_qT.tile([P, S], mdt, tag="qT", name="qT") for _ in range(cfg.nt_f)]
            kT_t = [sb_kT.tile([P, S], mdt, tag="kT", name="kT") for _ in range(cfg.nt_f)]
            v_t = [sb_v.tile([P, cfg.Hc * (Dh + 1)], mdt, tag="v", name="v") for _ in range(cfg.nt_s)]

            # ---------------- Phase 1: Q/K/V projections ----------------
            # x^T is streamed per 512-column s-chunk so the first matmuls
            # start ~2 MiB into the DMA instead of after the full 8 MiB.
            SCH = min(512, S)
            n_sch = S // SCH
            for _rep in range(getattr(cfg, "reps", 1)):
              with (
                tc.tile_pool(name=f"sb_xt{_rep}", bufs=2 * cfg.nt_d) as sb_xt,
                tc.tile_pool(name=f"sb_w{_rep}", bufs=3 * cfg.nt_d) as sb_w,
              ):
                def _wload(wdram):
                    lst = []
                    for d in range(cfg.nt_d):
                        t = sb_w.tile([P, F], mdt, tag="w", name="w")
                        nc.sync.dma_start(t[:], wdram[d * P:(d + 1) * P, :])
                        lst.append(t)
                    return lst

                def _xload(c):
                    lst = []
                    for d in range(cfg.nt_d):
                        t = sb_xt.tile([P, SCH], mdt, tag="xt", name="xt")
                        nc.sync.dma_start(
                            t[:], xT[d * P:(d + 1) * P, c * SCH:(c + 1) * SCH])
                        lst.append(t)
                    return lst

                # first-needed data first: wq, x chunk 0, then wk/wv
                wq_t = _wload(wqT)
                xt0 = _xload(0)
                wk_t = _wload(wkT)
                wv_t = _wload(wvT)

                for c in range(n_sch):
                    xt = xt0 if c == 0 else _xload(c)
                    # q^T / k^T columns for this chunk
                    for wt, dstT in ((wq_t, qT_t), (wk_t, kT_t)):
                        for m in range(cfg.nt_f):
                            ps = psA.tile([P, SCH], f32, tag="psA", name="ps")
                            for d in range(cfg.nt_d):
                                nc.tensor.matmul(
                                    ps[:],
                                    wt[d][:, m * P:(m + 1) * P],
                                    xt[d][:],
                                    start=(d == 0), stop=(d == cfg.nt_d - 1),
                                )
                            nc.vector.tensor_copy(
                                dstT[m][:, c * SCH:(c + 1) * SCH], ps[:])
                    # v rows for this chunk's s-tiles (natural layout plus an
                    # appended ones column per head: [64 features | 1] x Hc)
                    for u in range(SCH // P):
                        st = c * (SCH // P) + u
                        ps = psA.tile([P, F], f32, tag="psA", name="ps")
                        for d in range(cfg.nt_d):
                            nc.tensor.matmul(
                                ps[:],
                                xt[d][:, u * P:(u + 1) * P],
                                wv_t[d][:],
                                start=(d == 0), stop=(d == cfg.nt_d - 1),
                            )
                        dst = v_t[st][:].rearrange("p (h e) -> p h e", e=Dh + 1)
                        nc.vector.tensor_copy(
                            dst[:, :, 0:Dh],
                            ps[:].rearrange("p (h e) -> p h e", e=Dh),
                        )
                        nc.vector.tensor_copy(
                            dst[:, :, Dh:Dh + 1],
                            ones_c[:, None, 0:1].to_broadcast([P, cfg.Hc, 1]))

              # ---------------- Phase 2+3: attention + output proj ----------
              with (
                  tc.tile_pool(name=f"sb_ctx{_rep}", bufs=cfg.nt_f) as sb_ctx,
                  tc.tile_pool(name=f"sb_wo{_rep}", bufs=cfg.nt_f) as sb_wo,
                  tc.tile_pool(name=f"sb_exp{_rep}",
                               bufs=(8 if QCH <= 512 else 4)) as sb_exp,
                  tc.tile_pool(name=f"sb_out{_rep}",
                               bufs=(3 if QCH <= 512 else 2)) as sb_out,
                  tc.tile_pool(name=f"sb_rc{_rep}",
                               bufs=(4 if QCH <= 512 else 2)) as sb_rc,
              ):
                  ctxT_t = [sb_ctx.tile([P, S], mdt, tag="ctxT", name="ctxT") for _ in range(cfg.nt_f)]
                  wo_t = []
                  for f in range(cfg.nt_f):
                      t = sb_wo.tile([P, D], mdt, tag="wo")
                      nc.sync.dma_start(t[:], woT[f * P:(f + 1) * P, :])
                      wo_t.append(t)

                  MMW = min(512, QCH)   # max fp32 matmul free width (1 bank)
                  def _emit_wo(c, us=None):
                      # output projection for chunk c's rows
                      for u in (range(cfg.qt_per_ch) if us is None else us):
                          st = c * cfg.qt_per_ch + u
                          ot = sb_out.tile([P, D], f32, tag="ot", name="ot")
                          dw = min(512, D)
                          for dch in range(D // dw):
                              pwo = (psC if cfg.qch <= 512 else psA).tile(
                                  [P, dw], f32,
                                  tag=("pwo" if cfg.qch <= 512 else "psA"),
                                  name="pwo")
                              for f2 in range(cfg.nt_f):
                                  nc.tensor.matmul(
                                      pwo[:],
                                      ctxT_t[f2][:, st * P:(st + 1) * P],
                                      wo_t[f2][:, dch * dw:(dch + 1) * dw],
                                      start=(f2 == 0), stop=(f2 == cfg.nt_f - 1),
                                  )
                              nc.vector.tensor_copy(
                                  ot[:, dch * dw:(dch + 1) * dw], pwo[:])
                          nc.sync.dma_start(out[st * P:(st + 1) * P, :], ot[:])

                  # spread the delayed output projection of chunk c-1
                  # across chunk c's heads to fill ScalarE-wait gaps
                  wo_sched = {}
                  for u in range(cfg.qt_per_ch):
                      hs = min(cfg.Hc - 1,
                               max(1, int((u + 0.5) * cfg.Hc / cfg.qt_per_ch)))
                      wo_sched.setdefault(hs, []).append(u)
                  wo_sched = {h_: tuple(us_) for h_, us_ in wo_sched.items()}

                  for c in range(cfg.nqc):
                      ktiles = cfg.qt_per_ch * (c + 1)
                      for h in range(cfg.Hc):
                          if c > 0 and h in wo_sched:
                              _emit_wo(c - 1, us=wo_sched[h])  # gap filler
                          f, r = divmod(h, cfg.heads_per_ft)
                          rows = slice(r * Dh, (r + 1) * Dh)
                          pav = psB.tile([Dh + 1, QCH], f32, tag="pav")
                          for t in range(ktiles):
                              j = t - cfg.qt_per_ch * c
                              col0 = max(0, j * P)
                              pss = psA.tile([P, QCH], f32, tag="psA", name="pss")
                              for half in range(QCH // MMW):
                                  lo = max(col0, half * MMW)
                                  hi = (half + 1) * MMW
                                  if lo >= hi:
                                      continue
                                  nc.tensor.matmul(
                                      pss[:, lo:hi],
                                      kT_t[f][rows, t * P:(t + 1) * P],
                                      qT_t[f][rows, c * QCH + lo:c * QCH + hi],
                                      start=True, stop=True,
                                      tile_position=(r * Dh, 0),
                                  )
                              et = sb_exp.tile([P, QCH], mdt, tag="exp")
                              nc.scalar.activation(
                                  et[:, col0:], pss[:, col0:], Exp,
                                  bias=pb[:, t:t + 1], scale=float(Dh) ** -0.5,
                              )
                              if j >= 0:
                                  nc.vector.tensor_tensor(
                                      et[:, col0:col0 + P],
                                      et[:, col0:col0 + P], tri[:], mult)
                              for half in range(QCH // MMW):
                                  lo = max(col0, half * MMW)
                                  hi = (half + 1) * MMW
                                  if lo >= hi:
                                      continue
                                  nc.tensor.matmul(
                                      pav[:, lo:hi],
                                      v_t[t][:, h * (Dh + 1):(h + 1) * (Dh + 1)],
                                      et[:, lo:hi],
                                      start=(t == 0), stop=(t == ktiles - 1),
                                  )
                          rc = sb_rc.tile([1, QCH], f32, tag="rc")
                          rcb = sb_rc.tile([Dh, QCH], f32, tag="rcb")
                          nc.vector.reciprocal(rc[:], pav[Dh:Dh + 1, :])
                          nc.gpsimd.partition_broadcast(rcb[:], rc[:])
                          nc.vector.tensor_tensor(
                              ctxT_t[f][rows, c * QCH:(c + 1) * QCH],
                              pav[0:Dh, :], rcb[:], mult)

                      if c == cfg.nqc - 1:
                          _emit_wo(c)

    nc.compile()
    return nc


_NC_CACHE = {}


def _get_nc(cfg: Cfg):
    key = (cfg.B, cfg.S, cfg.D, cfg.H, cfg.n_cores, cfg.qch, cfg.mm_dtype, cfg.reps)
    if key not in _NC_CACHE:
        _NC_CACHE[key] = build_nc(cfg)
    return _NC_CACHE[key]


def make_in_maps(cfg: Cfg, x_self, padding_mask, Wq, Wk, Wv, Wo):
    """Host-side sharding: slice + transpose per core."""
    rnd = _round_f32r if cfg.mm_dtype == "fp32r" else (
        lambda a: np.ascontiguousarray(a, dtype=np.float32))
    in_maps = []
    for core in range(cfg.n_cores):
        b, g = divmod(core, cfg.groups)
        fsl = slice(g * cfg.F, (g + 1) * cfg.F)
        pbias = np.where(padding_mask[b], np.float32(NEG), np.float32(0.0))
        in_maps.append({
            "xT": rnd(x_self[b].T),
            "wqT": rnd(Wq[fsl, :].T),
            "wkT": rnd(Wk[fsl, :].T),
            "wvT": rnd(Wv[fsl, :].T),
            "woT": rnd(Wo[:, fsl].T),
            "pbias": np.ascontiguousarray(
                pbias.reshape(cfg.nt_s, P).T).astype(np.float32),
        })
    return in_maps


def kernel(x_self, x_other, padding_mask, Wq, Wk, Wv, Wo, _trace=False):
    x_self = np.asarray(x_self, dtype=np.float32)
    padding_mask = np.asarray(padding_mask)
    Wq = np.asarray(Wq, dtype=np.float32)
    Wk = np.asarray(Wk, dtype=np.float32)
    Wv = np.asarray(Wv, dtype=np.float32)
    Wo = np.asarray(Wo, dtype=np.float32)

    B, S, D = x_self.shape
    cfg = Cfg(B=B, S=S, D=D)
    nc = _get_nc(cfg)
    in_maps = make_in_maps(cfg, x_self, padding_mask, Wq, Wk, Wv, Wo)
    res = run_bass_kernel_spmd(
        nc, in_maps, core_ids=list(range(cfg.n_cores)), trace=_trace)

    out = np.zeros((B, S, D), dtype=np.float32)
    for core in range(cfg.n_cores):
        b = core // cfg.groups
        out[b] += res.results[core]["out"]
    if _trace:
        kernel.last_exec_time_ns = res.exec_time_ns
        kernel.last_results = res
    return out



# revision 2
# speedup vs baseline: 1.0524x; 1.0524x over previous
"""Multi-head causal self-attention on 8 Trainium2 NeuronCores.

Problem: B=4, S=2048, D=1024, H=16 heads (Dh=64), fp32, causal + key-padding
mask, out = softmax(mask(QK^T/sqrt(Dh))) V Wo^T with Q/K/V = x @ W*^T.

Sharding (data-parallel over batch x tensor-parallel over heads):
  core = 2*b + g  (b in 0..3, g in 0..1): batch b, head group g (8 heads).
  Each core computes its 8 heads' attention and a partial output projection
  through its row-slice of Wo; the host sums the two partials per batch
  (the "all-reduce" of the hint, done on host since outputs are gathered
  anyway).

Per-core kernel layout (everything f32; matmuls in float32r):
  - x^T [D, S] so projections contract D on partitions.
  - q^T, k^T stored [F=512, S] (head-major rows, 64 rows per head; heads
    2f/2f+1 live in partitions 0-63 / 64-127 of feature tile f).
  - scores computed TRANSPOSED per head: s^T[k, q] = k^T_tile.T @ q^T so the
    softmaxed tile feeds the AV matmul directly as the moving operand.
  - exp via ScalarE with fused scale (1/8) and per-key padding bias.
  - causal: only lower block-triangle computed; diagonal 128x128 blocks get a
    multiplicative triangular mask after exp; partial-width matmuls skip
    fully-masked column ranges.
  - V carries an appended ones-column per head so the AV matmul also yields
    the softmax denominators (row 64 of the [65, q] psum tile).
  - normalize: reciprocal on DVE, partition-broadcast on GpSimd, multiply on
    DVE straight into ctx^T tiles, which are the stationary operand of the
    output projection out[s, d] = ctx^T.T @ Wo_slice^T.
"""

import os
import numpy as np

import concourse.bass as bass
import concourse.mybir as mybir
import concourse.tile as tile
from concourse import bacc
from concourse.bass_utils import run_bass_kernel_spmd

P = 128
NEG = -1.0e30


def _round_f32r(a: np.ndarray) -> np.ndarray:
    """Round fp32 values to the PE's fp32r grid (11-bit mantissa,
    round-half-to-even at bit 12) so DMA-loaded tiles hold valid fp32r
    values. Matches walrus fp32_to_fp32r bit-exactly."""
    bits = np.ascontiguousarray(a, dtype=np.float32).view(np.uint32)
    low = bits & np.uint32(0xFFF)
    hi = bits & np.uint32(0xFFFFF000)
    add = (low > 0x800) | ((low == 0x800) & (((bits >> 12) & 1) == 1))
    return (hi + (add.astype(np.uint32) << 12)).view(np.float32)


class Cfg:
    def __init__(self, B=4, S=2048, D=1024, H=16, Dh=64, n_cores=8, qch=512,
                 mm_dtype="fp32r", reps=1):
        self.reps = reps
        self.B, self.S, self.D, self.H, self.Dh = B, S, D, H, Dh
        self.n_cores = n_cores
        self.groups = n_cores // B              # head groups (tensor-parallel)
        self.Hc = H // self.groups              # heads per core
        self.F = self.Hc * Dh                   # per-core q/k/v feature width
        self.qch = qch                          # q columns per score matmul
        self.nqc = S // qch                     # q chunks
        self.qt_per_ch = qch // P               # 128-row q tiles per chunk
        self.nt_s = S // P                      # key/seq tiles
        self.nt_d = D // P                      # contraction tiles (D)
        self.nt_f = self.F // P                 # feature tiles
        self.heads_per_ft = P // Dh             # heads packed per feature tile
        self.mm_dtype = mm_dtype

    @property
    def mdt(self):
        return {"fp32r": mybir.dt.float32r,
                "fp32": mybir.dt.float32,
                "bf16": mybir.dt.bfloat16}[self.mm_dtype]


def build_nc(cfg: Cfg):
    f32 = mybir.dt.float32
    mdt = cfg.mdt
    S, D, F, Dh = cfg.S, cfg.D, cfg.F, cfg.Dh
    QCH = cfg.qch

    nc = bacc.Bacc("TRN2", target_bir_lowering=False, debug=False,
                   num_devices=cfg.n_cores)

    xT = nc.dram_tensor("xT", [D, S], mdt, kind="ExternalInput").ap()
    wqT = nc.dram_tensor("wqT", [D, F], mdt, kind="ExternalInput").ap()
    wkT = nc.dram_tensor("wkT", [D, F], mdt, kind="ExternalInput").ap()
    wvT = nc.dram_tensor("wvT", [D, F], mdt, kind="ExternalInput").ap()
    woT = nc.dram_tensor("woT", [F, D], mdt, kind="ExternalInput").ap()
    pbias = nc.dram_tensor("pbias", [P, cfg.nt_s], f32, kind="ExternalInput").ap()
    out = nc.dram_tensor("out", [S, D], f32, kind="ExternalOutput").ap()

    Exp = mybir.ActivationFunctionType.Exp
    mult = mybir.AluOpType.mult

    with tile.TileContext(nc) as tc:
        with (
            tc.tile_pool(name="psA", bufs=(4 if cfg.qch <= 512 else 2),
                         space="PSUM") as psA,
            tc.tile_pool(name="psB", bufs=2, space="PSUM") as psB,
            tc.tile_pool(name="psC", bufs=(2 if cfg.qch <= 512 else 1),
                         space="PSUM") as psC,
            tc.tile_pool(name="sb_qT", bufs=cfg.nt_f) as sb_qT,
            tc.tile_pool(name="sb_kT", bufs=cfg.nt_f) as sb_kT,
            tc.tile_pool(name="sb_v", bufs=cfg.nt_s) as sb_v,
            tc.tile_pool(name="sb_misc", bufs=1) as sb_misc,
        ):
            # --- constants ---
            pb = sb_misc.tile([P, cfg.nt_s], f32, tag="pbias")
            nc.sync.dma_start(pb[:], pbias)
            # triangular keep-mask in [k(part), q(free)] coords: 1 where q>=k
            tri_f = sb_misc.tile([P, P], f32, tag="tri_f")
            nc.gpsimd.memset(tri_f[:], 1.0)
            nc.gpsimd.affine_select(
                out=tri_f[:], in_=tri_f[:],
                compare_op=mybir.AluOpType.is_ge, fill=0.0,
                base=0, channel_multiplier=-1, pattern=[[1, P]],
            )
            tri = sb_misc.tile([P, P], mdt, tag="tri")
            nc.vector.tensor_copy(tri[:], tri_f[:])
            ones_c = sb_misc.tile([P, 1], f32, tag="ones_c")
            nc.gpsimd.memset(ones_c[:], 1.0)

            qT_t = [sb_qT.tile([P, S], mdt, tag="qT", name="qT") for _ in range(cfg.nt_f)]
            kT_t = [sb_kT.tile([P, S], mdt, tag="kT", name="kT") for _ in range(cfg.nt_f)]
            v_t = [sb_v.tile([P, cfg.Hc * (Dh + 1)], mdt, tag="v", name="v") for _ in range(cfg.nt_s)]

            # ---------------- Phase 1: Q/K/V projections ----------------
            # x^T is streamed per 512-column s-chunk so the first matmuls
            # start ~2 MiB into the DMA instead of after the full 8 MiB.
            SCH = min(512, S)
            n_sch = S // SCH
            for _rep in range(getattr(cfg, "reps", 1)):
              with (
                tc.tile_pool(name=f"sb_xt{_rep}", bufs=2 * cfg.nt_d) as sb_xt,
                tc.tile_pool(name=f"sb_w{_rep}", bufs=3 * cfg.nt_d) as sb_w,
              ):
                def _wload(wdram):
                    lst = []
                    for d in range(cfg.nt_d):
                        t = sb_w.tile([P, F], mdt, tag="w", name="w")
                        nc.sync.dma_start(t[:], wdram[d * P:(d + 1) * P, :])
                        lst.append(t)
                    return lst

                def _xload(c):
                    lst = []
                    for d in range(cfg.nt_d):
                        t = sb_xt.tile([P, SCH], mdt, tag="xt", name="xt")
                        nc.sync.dma_start(
                            t[:], xT[d * P:(d + 1) * P, c * SCH:(c + 1) * SCH])
                        lst.append(t)
                    return lst

                # first-needed data first: wq, x chunk 0, then wk/wv
                wq_t = _wload(wqT)
                xt0 = _xload(0)
                wk_t = _wload(wkT)
                wv_t = _wload(wvT)

                for c in range(n_sch):
                    xt = xt0 if c == 0 else _xload(c)
                    # q^T / k^T columns for this chunk
                    for wt, dstT in ((wq_t, qT_t), (wk_t, kT_t)):
                        for m in range(cfg.nt_f):
                            ps = psA.tile([P, SCH], f32, tag="psA", name="ps")
                            for d in range(cfg.nt_d):
                                nc.tensor.matmul(
                                    ps[:],
                                    wt[d][:, m * P:(m + 1) * P],
                                    xt[d][:],
                                    start=(d == 0), stop=(d == cfg.nt_d - 1),
                                )
                            nc.vector.tensor_copy(
                                dstT[m][:, c * SCH:(c + 1) * SCH], ps[:])
                    # v rows for this chunk's s-tiles (natural layout plus an
                    # appended ones column per head: [64 features | 1] x Hc)
                    for u in range(SCH // P):
                        st = c * (SCH // P) + u
                        ps = psA.tile([P, F], f32, tag="psA", name="ps")
                        for d in range(cfg.nt_d):
                            nc.tensor.matmul(
                                ps[:],
                                xt[d][:, u * P:(u + 1) * P],
                                wv_t[d][:],
                                start=(d == 0), stop=(d == cfg.nt_d - 1),
                            )
                        dst = v_t[st][:].rearrange("p (h e) -> p h e", e=Dh + 1)
                        nc.vector.tensor_copy(
                            dst[:, :, 0:Dh],
                            ps[:].rearrange("p (h e) -> p h e", e=Dh),
                        )
                        nc.vector.tensor_copy(
                            dst[:, :, Dh:Dh + 1],
                            ones_c[:, None, 0:1].to_broadcast([P, cfg.Hc, 1]))

              # ---------------- Phase 2+3: attention + output proj ----------
              with (
                  tc.tile_pool(name=f"sb_ctx{_rep}", bufs=cfg.nt_f) as sb_ctx,
                  tc.tile_pool(name=f"sb_wo{_rep}", bufs=cfg.nt_f) as sb_wo,
                  tc.tile_pool(name=f"sb_exp{_rep}",
                               bufs=(8 if QCH <= 512 else 4)) as sb_exp,
                  tc.tile_pool(name=f"sb_out{_rep}",
                               bufs=(3 if QCH <= 512 else 2)) as sb_out,
                  tc.tile_pool(name=f"sb_rc{_rep}",
                               bufs=(4 if QCH <= 512 else 2)) as sb_rc,
              ):
                  ctxT_t = [sb_ctx.tile([P, S], mdt, tag="ctxT", name="ctxT") for _ in range(cfg.nt_f)]
                  wo_t = []
                  for f in range(cfg.nt_f):
                      t = sb_wo.tile([P, D], mdt, tag="wo")
                      nc.sync.dma_start(t[:], woT[f * P:(f + 1) * P, :])
                      wo_t.append(t)

                  MMW = min(512, QCH)   # max fp32 matmul free width (1 bank)
                  def _emit_wo(c, us=None):
                      # output projection for chunk c's rows
                      for u in (range(cfg.qt_per_ch) if us is None else us):
                          st = c * cfg.qt_per_ch + u
                          ot = sb_out.tile([P, D], f32, tag="ot", name="ot")
                          dw = min(512, D)
                          for dch in range(D // dw):
                              pwo = (psC if cfg.qch <= 512 else psA).tile(
                                  [P, dw], f32,
                                  tag=("pwo" if cfg.qch <= 512 else "psA"),
                                  name="pwo")
                              for f2 in range(cfg.nt_f):
                                  nc.tensor.matmul(
                                      pwo[:],
                                      ctxT_t[f2][:, st * P:(st + 1) * P],
                                      wo_t[f2][:, dch * dw:(dch + 1) * dw],
                                      start=(f2 == 0), stop=(f2 == cfg.nt_f - 1),
                                  )
                              nc.vector.tensor_copy(
                                  ot[:, dch * dw:(dch + 1) * dw], pwo[:])
                          nc.sync.dma_start(out[st * P:(st + 1) * P, :], ot[:])

                  # Heads are processed in PAIRS (2f, 2f+1) living in rows
                  # 0-63 / 64-127 of feature tile f. Their score matmuls are
                  # issued back-to-back at tile_position (0,0)/(64,0): the PE
                  # executes row-disjoint quadrant matmuls concurrently and
                  # overlaps their weight loads, ~4x faster than same-quadrant
                  # (measured: 110ns vs 520ns per 512-wide 64-contract mm).
                  # The AV matmuls trail the scores by one k-tile so the PE
                  # never waits on the Act engine's exp.
                  for c in range(cfg.nqc):
                      ktiles = cfg.qt_per_ch * (c + 1)
                      for f in range(cfg.nt_f):
                          if c > 0:
                              _emit_wo(c - 1, us=(f,))  # gap filler
                          rA = slice(0, Dh)
                          rB = slice(Dh, 2 * Dh)
                          hA = 2 * f
                          hB = 2 * f + 1
                          pavA = psB.tile([Dh + 1, QCH], f32, tag="pav")
                          pavB = psB.tile([Dh + 1, QCH], f32, tag="pav")
                          prev = None
                          for t in range(ktiles):
                              j = t - cfg.qt_per_ch * c
                              col0 = max(0, j * P)
                              pssA = psA.tile([P, QCH], f32, tag="psA", name="pss")
                              pssB = psA.tile([P, QCH], f32, tag="psA", name="pss")
                              nc.tensor.matmul(
                                  pssA[:, col0:],
                                  kT_t[f][rA, t * P:(t + 1) * P],
                                  qT_t[f][rA, c * QCH + col0:(c + 1) * QCH],
                                  start=True, stop=True,
                                  tile_position=(0, 0),
                              )
                              nc.tensor.matmul(
                                  pssB[:, col0:],
                                  kT_t[f][rB, t * P:(t + 1) * P],
                                  qT_t[f][rB, c * QCH + col0:(c + 1) * QCH],
                                  start=True, stop=True,
                                  tile_position=(Dh, 0),
                              )
                              etA = sb_exp.tile([P, QCH], mdt, tag="exp")
                              etB = sb_exp.tile([P, QCH], mdt, tag="exp")
                              nc.scalar.activation(
                                  etA[:, col0:], pssA[:, col0:], Exp,
                                  bias=pb[:, t:t + 1], scale=float(Dh) ** -0.5,
                              )
                              nc.scalar.activation(
                                  etB[:, col0:], pssB[:, col0:], Exp,
                                  bias=pb[:, t:t + 1], scale=float(Dh) ** -0.5,
                              )
                              if j >= 0:
                                  nc.vector.tensor_tensor(
                                      etA[:, col0:col0 + P],
                                      etA[:, col0:col0 + P], tri[:], mult)
                                  nc.vector.tensor_tensor(
                                      etB[:, col0:col0 + P],
                                      etB[:, col0:col0 + P], tri[:], mult)
                              if prev is not None:
                                  pt, pcol0, petA, petB = prev
                                  nc.tensor.matmul(
                                      pavA[:, pcol0:],
                                      v_t[pt][:, hA * (Dh + 1):(hA + 1) * (Dh + 1)],
                                      petA[:, pcol0:],
                                      start=(pt == 0), stop=False,
                                  )
                                  nc.tensor.matmul(
                                      pavB[:, pcol0:],
                                      v_t[pt][:, hB * (Dh + 1):(hB + 1) * (Dh + 1)],
                                      petB[:, pcol0:],
                                      start=(pt == 0), stop=False,
                                  )
                              prev = (t, col0, etA, etB)
                          pt, pcol0, petA, petB = prev
                          nc.tensor.matmul(
                              pavA[:, pcol0:],
                              v_t[pt][:, hA * (Dh + 1):(hA + 1) * (Dh + 1)],
                              petA[:, pcol0:],
                              start=(pt == 0), stop=True,
                          )
                          nc.tensor.matmul(
                              pavB[:, pcol0:],
                              v_t[pt][:, hB * (Dh + 1):(hB + 1) * (Dh + 1)],
                              petB[:, pcol0:],
                              start=(pt == 0), stop=True,
                          )
                          for pav, rows in ((pavA, rA), (pavB, rB)):
                              rc = sb_rc.tile([1, QCH], f32, tag="rc")
                              rcb = sb_rc.tile([Dh, QCH], f32, tag="rcb")
                              nc.vector.reciprocal(rc[:], pav[Dh:Dh + 1, :])
                              nc.gpsimd.partition_broadcast(rcb[:], rc[:])
                              nc.vector.tensor_tensor(
                                  ctxT_t[f][rows, c * QCH:(c + 1) * QCH],
                                  pav[0:Dh, :], rcb[:], mult)

                      if c == cfg.nqc - 1:
                          _emit_wo(c)

    nc.compile()
    return nc


_NC_CACHE = {}


def _get_nc(cfg: Cfg):
    key = (cfg.B, cfg.S, cfg.D, cfg.H, cfg.n_cores, cfg.qch, cfg.mm_dtype, cfg.reps)
    if key not in _NC_CACHE:
        _NC_CACHE[key] = build_nc(cfg)
    return _NC_CACHE[key]


def make_in_maps(cfg: Cfg, x_self, padding_mask, Wq, Wk, Wv, Wo):
    """Host-side sharding: slice + transpose per core."""
    rnd = _round_f32r if cfg.mm_dtype == "fp32r" else (
        lambda a: np.ascontiguousarray(a, dtype=np.float32))
    in_maps = []
    for core in range(cfg.n_cores):
        b, g = divmod(core, cfg.groups)
        fsl = slice(g * cfg.F, (g + 1) * cfg.F)
        pbias = np.where(padding_mask[b], np.float32(NEG), np.float32(0.0))
        in_maps.append({
            "xT": rnd(x_self[b].T),
            "wqT": rnd(Wq[fsl, :].T),
            "wkT": rnd(Wk[fsl, :].T),
            "wvT": rnd(Wv[fsl, :].T),
            "woT": rnd(Wo[:, fsl].T),
            "pbias": np.ascontiguousarray(
                pbias.reshape(cfg.nt_s, P).T).astype(np.float32),
        })
    return in_maps


def kernel(x_self, x_other, padding_mask, Wq, Wk, Wv, Wo, _trace=False):
    x_self = np.asarray(x_self, dtype=np.float32)
    padding_mask = np.asarray(padding_mask)
    Wq = np.asarray(Wq, dtype=np.float32)
    Wk = np.asarray(Wk, dtype=np.float32)
    Wv = np.asarray(Wv, dtype=np.float32)
    Wo = np.asarray(Wo, dtype=np.float32)

    B, S, D = x_self.shape
    cfg = Cfg(B=B, S=S, D=D)
    nc = _get_nc(cfg)
    in_maps = make_in_maps(cfg, x_self, padding_mask, Wq, Wk, Wv, Wo)
    res = run_bass_kernel_spmd(
        nc, in_maps, core_ids=list(range(cfg.n_cores)), trace=_trace)

    out = np.zeros((B, S, D), dtype=np.float32)
    for core in range(cfg.n_cores):
        b = core // cfg.groups
        out[b] += res.results[core]["out"]
    if _trace:
        kernel.last_exec_time_ns = res.exec_time_ns
        kernel.last_results = res
    return out



# revision 4
# speedup vs baseline: 1.9524x; 1.8551x over previous
"""Multi-head causal self-attention on 8 Trainium2 NeuronCores.

Problem: B=4, S=2048, D=1024, H=16 heads (Dh=64), fp32, causal + key-padding
mask, out = softmax(mask(QK^T/sqrt(Dh))) V Wo^T with Q/K/V = x @ W*^T.

Sharding (data-parallel over batch x tensor-parallel over heads):
  core = 2*b + g  (b in 0..3, g in 0..1): batch b, head group g (8 heads).
  Each core computes its 8 heads' attention and a partial output projection
  through its row-slice of Wo; the host sums the two partials per batch.

Schedule (v5): fully chunk-interleaved.
  - proj(0) head; then for each q-chunk c: attention tile-groups of chunk c
    interleaved with "filler" granules = proj(c+1) + wo(c-1); wo(3) tail.
  - Head PAIRS (2f, 2f+1) in rows 0-63/64-127 of feature tile f issue their
    score matmuls back-to-back at tile_position (0,0)/(64,0): the PE runs
    row-disjoint quadrant matmuls concurrently (~110ns vs 520ns per 512-wide
    64-contract matmul, measured).
  - Both heads' scores land in one [P, 2*QCH] psum pair-tile -> a single
    fused exp (Act) and a single fused tri-mask (DVE) per k-tile.
  - AV matmuls trail the scores by one k-tile so the PE never waits on exp.
  - V carries an appended ones-column per head so AV also yields the softmax
    denominators (row 64 of the [65, q] psum tile).
"""

import os
import numpy as np

import concourse.bass as bass
import concourse.mybir as mybir
import concourse.tile as tile
from concourse import bacc
from concourse.bass_utils import run_bass_kernel_spmd

P = 128
NEG = -1.0e30


def _round_f32r(a: np.ndarray) -> np.ndarray:
    bits = np.ascontiguousarray(a, dtype=np.float32).view(np.uint32)
    low = bits & np.uint32(0xFFF)
    hi = bits & np.uint32(0xFFFFF000)
    add = (low > 0x800) | ((low == 0x800) & (((bits >> 12) & 1) == 1))
    return (hi + (add.astype(np.uint32) << 12)).view(np.float32)


class Cfg:
    def __init__(self, B=4, S=2048, D=1024, H=16, Dh=64, n_cores=8, qch=512,
                 mm_dtype="bf16", reps=1):
        self.reps = reps
        self.B, self.S, self.D, self.H, self.Dh = B, S, D, H, Dh
        self.n_cores = n_cores
        self.groups = n_cores // B
        self.Hc = H // self.groups
        self.F = self.Hc * Dh
        self.qch = qch
        self.nqc = S // qch
        self.qt_per_ch = qch // P
        self.nt_s = S // P
        self.nt_d = D // P
        self.nt_f = self.F // P
        self.heads_per_ft = P // Dh
        self.mm_dtype = mm_dtype

    @property
    def mdt(self):
        return {"fp32r": mybir.dt.float32r,
                "fp32": mybir.dt.float32,
                "bf16": mybir.dt.bfloat16}[self.mm_dtype]


def build_nc(cfg: Cfg):
    f32 = mybir.dt.float32
    mdt = cfg.mdt
    S, D, F, Dh = cfg.S, cfg.D, cfg.F, cfg.Dh
    QCH = cfg.qch
    assert QCH == 512

    nc = bacc.Bacc("TRN2", target_bir_lowering=False, debug=False,
                   num_devices=cfg.n_cores)

    xT = nc.dram_tensor("xT", [D, S], mdt, kind="ExternalInput").ap()
    wqT = nc.dram_tensor("wqT", [D, F], mdt, kind="ExternalInput").ap()
    wkT = nc.dram_tensor("wkT", [D, F], mdt, kind="ExternalInput").ap()
    wvT = nc.dram_tensor("wvT", [D, F], mdt, kind="ExternalInput").ap()
    woT = nc.dram_tensor("woT", [F, D], mdt, kind="ExternalInput").ap()
    pbias = nc.dram_tensor("pbias", [P, cfg.nt_s], f32, kind="ExternalInput").ap()
    out = nc.dram_tensor("out", [S, D], f32, kind="ExternalOutput").ap()

    Exp = mybir.ActivationFunctionType.Exp
    mult = mybir.AluOpType.mult

    with tile.TileContext(nc) as tc:
        with (
            tc.tile_pool(name="psA", bufs=2, space="PSUM") as psA,   # [P,2*QCH]
            tc.tile_pool(name="psB", bufs=2, space="PSUM") as psB,   # pav
            tc.tile_pool(name="psC", bufs=2, space="PSUM") as psC,   # proj/wo
            tc.tile_pool(name="sb_kT", bufs=cfg.nt_f) as sb_kT,
            tc.tile_pool(name="sb_v", bufs=cfg.nt_s) as sb_v,
            tc.tile_pool(name="sb_misc", bufs=1) as sb_misc,
        ):
            # --- constants ---
            pb = sb_misc.tile([P, cfg.nt_s], f32, tag="pbias")
            nc.sync.dma_start(pb[:], pbias)
            tri_f = sb_misc.tile([P, P], f32, tag="tri_f")
            nc.gpsimd.memset(tri_f[:], 1.0)
            nc.gpsimd.affine_select(
                out=tri_f[:], in_=tri_f[:],
                compare_op=mybir.AluOpType.is_ge, fill=0.0,
                base=0, channel_multiplier=-1, pattern=[[1, P]],
            )
            tri = sb_misc.tile([P, P], mdt, tag="tri")
            nc.vector.tensor_copy(tri[:], tri_f[:])
            ones_c = sb_misc.tile([P, 1], f32, tag="ones_c")
            nc.gpsimd.memset(ones_c[:], 1.0)

            kT_t = [sb_kT.tile([P, S], mdt, tag="kT", name="kT")
                    for _ in range(cfg.nt_f)]
            v_t = [sb_v.tile([P, cfg.Hc * (Dh + 1)], mdt, tag="v", name="v")
                   for _ in range(cfg.nt_s)]

            for _rep in range(cfg.reps):
              with (
                tc.tile_pool(name=f"sb_xt{_rep}", bufs=cfg.nt_d) as sb_xt,
                tc.tile_pool(name=f"sb_w{_rep}", bufs=3 * cfg.nt_d) as sb_w,
                tc.tile_pool(name=f"sb_wo{_rep}", bufs=cfg.nt_f) as sb_wo,
                tc.tile_pool(name=f"sb_qT{_rep}", bufs=2 * cfg.nt_f) as sb_qT,
                tc.tile_pool(name=f"sb_ctx{_rep}", bufs=4 * cfg.nt_f) as sb_ctx,
                tc.tile_pool(name=f"sb_exp{_rep}", bufs=8) as sb_exp,
                tc.tile_pool(name=f"sb_out{_rep}", bufs=3) as sb_out,
                tc.tile_pool(name=f"sb_rc{_rep}", bufs=4) as sb_rc,
                tc.tile_pool(name=f"sb_sn{_rep}", bufs=4) as sb_sn,
              ):
                def _wload(wdram, n=None, pool=None, width=None):
                    lst = []
                    for d in range(n or cfg.nt_d):
                        t = (pool or sb_w).tile([P, width or F], mdt, tag="w",
                                                name="w")
                        nc.sync.dma_start(t[:], wdram[d * P:(d + 1) * P, :])
                        lst.append(t)
                    return lst

                def _xload(c):
                    lst = []
                    for d in range(cfg.nt_d):
                        t = sb_xt.tile([P, QCH], mdt, tag="xt", name="xt")
                        nc.sync.dma_start(
                            t[:], xT[d * P:(d + 1) * P, c * QCH:(c + 1) * QCH])
                        lst.append(t)
                    return lst

                wq_t = _wload(wqT)
                xt = {0: _xload(0)}
                wk_t = _wload(wkT)
                wv_t = _wload(wvT)
                wo_t = _wload(woT, n=cfg.nt_f, pool=sb_wo, width=D)

                qT = {}    # c -> [nt_f tiles of [P, QCH]]
                ctx = {}   # c -> [nt_f tiles of [P, QCH]]

                def _proj_granules(c):
                    """Return a list of emitter thunks for chunk c's q/k/v."""
                    qT[c] = [sb_qT.tile([P, QCH], mdt, tag="qT", name="qT")
                             for _ in range(cfg.nt_f)]
                    gs = []

                    def _qk(wt, m, dst_tile, dst_cols):
                        def g():
                            ps = psC.tile([P, QCH], f32, tag="psC", name="ps")
                            for d in range(cfg.nt_d):
                                nc.tensor.matmul(
                                    ps[:],
                                    wt[d][:, m * P:(m + 1) * P],
                                    xt[c][d][:],
                                    start=(d == 0), stop=(d == cfg.nt_d - 1),
                                )
                            nc.vector.tensor_copy(dst_tile[:, dst_cols], ps[:])
                        return g

                    def _v(u):
                        def g():
                            st = c * cfg.qt_per_ch + u
                            ps = psC.tile([P, F], f32, tag="psC", name="ps")
                            for d in range(cfg.nt_d):
                                nc.tensor.matmul(
                                    ps[:],
                                    xt[c][d][:, u * P:(u + 1) * P],
                                    wv_t[d][:],
                                    start=(d == 0), stop=(d == cfg.nt_d - 1),
                                )
                            dst = v_t[st][:].rearrange("p (h e) -> p h e",
                                                       e=Dh + 1)
                            nc.vector.tensor_copy(
                                dst[:, :, 0:Dh],
                                ps[:].rearrange("p (h e) -> p h e", e=Dh),
                            )
                            nc.vector.tensor_copy(
                                dst[:, :, Dh:Dh + 1],
                                ones_c[:, None, 0:1].to_broadcast([P, cfg.Hc, 1]))
                        return g

                    # k granules last: the kT tile write WARs (tile-granular)
                    # on pair m's scores still reading kT_t[m] this chunk, so
                    # they should fire as late as possible.
                    for m in range(cfg.nt_f):
                        gs.append(_qk(wq_t, m, qT[c][m], slice(0, QCH)))
                    for u in range(cfg.qt_per_ch):
                        gs.append(_v(u))
                    for m in range(cfg.nt_f):
                        gs.append(_qk(wk_t, m, kT_t[m],
                                      slice(c * QCH, (c + 1) * QCH)))
                    return gs

                def _wo_granule(c, u):
                    def g():
                        st = c * cfg.qt_per_ch + u
                        ot = sb_out.tile([P, D], f32, tag="ot", name="ot")
                        for dch in range(D // 512):
                            pwo = psC.tile([P, 512], f32, tag="psC", name="pwo")
                            for f2 in range(cfg.nt_f):
                                nc.tensor.matmul(
                                    pwo[:],
                                    ctx[c][f2][:, u * P:(u + 1) * P],
                                    wo_t[f2][:, dch * 512:(dch + 1) * 512],
                                    start=(f2 == 0), stop=(f2 == cfg.nt_f - 1),
                                )
                            nc.vector.tensor_copy(
                                ot[:, dch * 512:(dch + 1) * 512], pwo[:])
                        nc.sync.dma_start(out[st * P:(st + 1) * P, :], ot[:])
                    return g

                def _attn_pair(c, f, fillers, fill_state):
                    """Attention for head pair (2f, 2f+1), chunk c; pops
                    filler thunks at a steady rate between tile-groups."""
                    ktiles = cfg.qt_per_ch * (c + 1)
                    rA, rB = slice(0, Dh), slice(Dh, 2 * Dh)
                    hA, hB = 2 * f, 2 * f + 1
                    pavA = psB.tile([Dh + 1, QCH], f32, tag="pav")
                    pavB = psB.tile([Dh + 1, QCH], f32, tag="pav")
                    pending = []
                    for t in range(ktiles):
                        j = t - cfg.qt_per_ch * c
                        col0 = max(0, j * P)
                        pss = psA.tile([P, 2 * QCH], f32, tag="psA", name="pss")
                        nc.tensor.matmul(
                            pss[:, col0:QCH],
                            kT_t[f][rA, t * P:(t + 1) * P],
                            qT[c][f][rA, col0:QCH],
                            start=True, stop=True, tile_position=(0, 0),
                        )
                        nc.tensor.matmul(
                            pss[:, QCH + col0:2 * QCH],
                            kT_t[f][rB, t * P:(t + 1) * P],
                            qT[c][f][rB, col0:QCH],
                            start=True, stop=True, tile_position=(Dh, 0),
                        )
                        etP = sb_exp.tile([P, 2, QCH], mdt, tag="exp")
                        pss2 = pss[:].rearrange("p (h w) -> p h w", h=2)
                        nc.scalar.activation(
                            etP[:, :, col0:], pss2[:, :, col0:], Exp,
                            bias=pb[:, t:t + 1], scale=float(Dh) ** -0.5,
                        )
                        if j >= 0:
                            nc.vector.tensor_tensor(
                                etP[:, :, col0:col0 + P],
                                etP[:, :, col0:col0 + P],
                                tri[:, None, :].to_broadcast([P, 2, P]), mult)
                        pending.append((t, col0, etP))
                        if len(pending) > 3:
                            pt, pcol0, petP = pending.pop(0)
                            nc.tensor.matmul(
                                pavA[:, pcol0:],
                                v_t[pt][:, hA * (Dh + 1):(hA + 1) * (Dh + 1)],
                                petP[:, 0, pcol0:],
                                start=(pt == 0), stop=False,
                            )
                            nc.tensor.matmul(
                                pavB[:, pcol0:],
                                v_t[pt][:, hB * (Dh + 1):(hB + 1) * (Dh + 1)],
                                petP[:, 1, pcol0:],
                                start=(pt == 0), stop=False,
                            )
                        # steady-rate filler emission
                        fill_state[0] += fill_state[1]
                        while fill_state[0] >= 1.0 and fillers:
                            fill_state[0] -= 1.0
                            fillers.pop(0)()
                    while pending:
                        pt, pcol0, petP = pending.pop(0)
                        last = not pending
                        nc.tensor.matmul(
                            pavA[:, pcol0:],
                            v_t[pt][:, hA * (Dh + 1):(hA + 1) * (Dh + 1)],
                            petP[:, 0, pcol0:],
                            start=(pt == 0), stop=last,
                        )
                        nc.tensor.matmul(
                            pavB[:, pcol0:],
                            v_t[pt][:, hB * (Dh + 1):(hB + 1) * (Dh + 1)],
                            petP[:, 1, pcol0:],
                            start=(pt == 0), stop=last,
                        )
                    for pav, rows in ((pavA, rA), (pavB, rB)):
                        sn = sb_sn.tile([Dh + 1, QCH], f32, tag="sn")
                        nc.vector.tensor_copy(sn[:], pav[:])
                        rc = sb_rc.tile([1, QCH], f32, tag="rc")
                        rcb = sb_rc.tile([Dh, QCH], f32, tag="rcb")
                        nc.vector.reciprocal(rc[:], sn[Dh:Dh + 1, :])
                        nc.gpsimd.partition_broadcast(rcb[:], rc[:])
                        nc.vector.tensor_tensor(
                            ctx[c][f][rows, :], sn[0:Dh, :], rcb[:], mult)

                # ---- head: chunk 0 projections, un-overlapped ----
                for g in _proj_granules(0):
                    g()

                for c in range(cfg.nqc):
                    ctx[c] = [sb_ctx.tile([P, QCH], mdt, tag="ctx", name="ctx")
                              for _ in range(cfg.nt_f)]
                    fillers = []
                    if c + 1 < cfg.nqc:
                        xt[c + 1] = _xload(c + 1)
                        fillers += _proj_granules(c + 1)
                    if c == cfg.nqc - 1:
                        for cc in range(cfg.nqc - 1):
                            fillers += [_wo_granule(cc, u)
                                        for u in range(cfg.qt_per_ch)]
                    n_groups = cfg.qt_per_ch * (c + 1) * cfg.nt_f
                    fill_state = [0.0, len(fillers) / n_groups]
                    for f in range(cfg.nt_f):
                        _attn_pair(c, f, fillers, fill_state)
                    for g in fillers:  # leftovers (rounding)
                        g()
                for u in range(cfg.qt_per_ch):
                    _wo_granule(cfg.nqc - 1, u)()

    nc.compile()
    return nc


_NC_CACHE = {}


def _get_nc(cfg: Cfg):
    key = (cfg.B, cfg.S, cfg.D, cfg.H, cfg.n_cores, cfg.qch, cfg.mm_dtype,
           cfg.reps)
    if key not in _NC_CACHE:
        _NC_CACHE[key] = build_nc(cfg)
    return _NC_CACHE[key]


def make_in_maps(cfg: Cfg, x_self, padding_mask, Wq, Wk, Wv, Wo):
    if cfg.mm_dtype == "fp32r":
        rnd = _round_f32r
    elif cfg.mm_dtype == "bf16":
        import ml_dtypes
        rnd = lambda a: np.ascontiguousarray(np.asarray(a, dtype=np.float32)).astype(ml_dtypes.bfloat16)
    else:
        rnd = lambda a: np.ascontiguousarray(a, dtype=np.float32)
    in_maps = []
    for core in range(cfg.n_cores):
        b, g = divmod(core, cfg.groups)
        fsl = slice(g * cfg.F, (g + 1) * cfg.F)
        pbias = np.where(padding_mask[b], np.float32(NEG), np.float32(0.0))
        in_maps.append({
            "xT": rnd(x_self[b].T),
            "wqT": rnd(Wq[fsl, :].T),
            "wkT": rnd(Wk[fsl, :].T),
            "wvT": rnd(Wv[fsl, :].T),
            "woT": rnd(Wo[:, fsl].T),
            "pbias": np.ascontiguousarray(
                pbias.reshape(cfg.nt_s, P).T).astype(np.float32),
        })
    return in_maps


def kernel(x_self, x_other, padding_mask, Wq, Wk, Wv, Wo, _trace=False):
    x_self = np.asarray(x_self, dtype=np.float32)
    padding_mask = np.asarray(padding_mask)
    Wq = np.asarray(Wq, dtype=np.float32)
    Wk = np.asarray(Wk, dtype=np.float32)
    Wv = np.asarray(Wv, dtype=np.float32)
    Wo = np.asarray(Wo, dtype=np.float32)

    B, S, D = x_self.shape
    cfg = Cfg(B=B, S=S, D=D)
    nc = _get_nc(cfg)
    in_maps = make_in_maps(cfg, x_self, padding_mask, Wq, Wk, Wv, Wo)
    res = run_bass_kernel_spmd(
        nc, in_maps, core_ids=list(range(cfg.n_cores)), trace=_trace)

    out = np.zeros((B, S, D), dtype=np.float32)
    for core in range(cfg.n_cores):
        b = core // cfg.groups
        out[b] += res.results[core]["out"]
    if _trace:
        kernel.last_exec_time_ns = res.exec_time_ns
        kernel.last_results = res
    return out
